# revision 29
# baseline (speedup 1.0000x reference)
"""Trainium2 Bass kernel for a DeepSpeed-style transformer encoder layer.

Strategy: data-parallel over 8 NeuronCores. Each core owns 1024 tokens
(half of one batch's sequence); K/V are computed redundantly for the full
2048-token sequence on each core, so there is no cross-core communication.
Odd cores receive their sequence rolled by 1024 so one SPMD program serves
all cores.

The attention path runs in fp8-e4m3 on the TensorEngine with weights
pre-scaled by 32 (so 0.02-magnitude weights land in fp8's normal range):
  - QKV / ctx / O-proj matmuls use DoubleRow perf mode (2 fp8 weights per
    PE cell -> 2x contraction throughput),
  - score matmuls pack the two heads of a 128-feature block into row
    groups 0-1 / 2-3 of the PE array (two concurrent K=64 matmuls via
    tile_position), recovering the half-array waste of a 64-dim head,
  - softmax probs come out of the scalar engine exp directly in fp8; the
    denominator rides along as a 65th "ones" column of V.
The FFN stays bf16 (fp8 there fails the accuracy budget).  All scale
factors cancel exactly: scores carry 32*32 which folds into the exp scale,
ctx carries 32 which cancels in the O-proj descale 1/(32*32).

The kernel is software-pipelined in two 512-query-token chunks: the
scalar-engine softmax exp of chunk c overlaps the PE-bound FFN2 of chunk
c-1 (emitted as filler between score/ctx matmuls - engines execute their
queues in order, so emission order is the schedule).  V projections fill
chunk 0 the same way.  Gelu runs in per-chunk bursts to avoid thrashing
the scalar engine's activation-table sets (exp and gelu live in different
table sets; a switch costs ~2.7us).
"""

import contextlib
import ctypes
import os
import sys
import types

import numpy as np
import ml_dtypes

B, S, H = 4, 2048, 1024
HEADS, HD, DFF, P = 16, 64, 4096, 128
NCORES = 8
TOK = 2048          # k/v token domain per core (full sequence)
OWN = 1024          # query tokens per core
EPS = 1e-12
WS = 32.0                       # fp8 weight pre-scale
SCALE = 1.0 / (8.0 * WS * WS)   # exp scale: 1/sqrt(HD) / (q,k weight scales)
OSC = 1.0 / (WS * WS)           # O-proj descale (ctx x32, wo x32)
CHUNKS = (512, 512)             # query-chunk widths (sum == OWN, each <= 512)

_CACHE = {}
LAST_EXEC_NS = None


# ---------------------------------------------------------------- trace hook
def _install_trace_hook():
    """Recreate the antenv.axon_hooks NTFF profile hook missing from this
    image, so run_bass_kernel_spmd(trace=True) works (used by test.py)."""
    if "antenv.axon_hooks" in sys.modules:
        return
    so_path = "/opt/axon/libaxon_pjrt.so"

    def _make(so):
        try:
            lib = ctypes.CDLL(so)
        except OSError:
            return None
        if not hasattr(lib, "axon_start_nrt_profile"):
            return None
        lib.axon_start_nrt_profile.argtypes = [
            ctypes.POINTER(ctypes.c_int64), ctypes.c_size_t]
        lib.axon_start_nrt_profile.restype = ctypes.c_int64
        lib.axon_stop_nrt_profile.argtypes = [ctypes.c_char_p]
        lib.axon_stop_nrt_profile.restype = ctypes.c_int64

        @contextlib.contextmanager
        def _hook(output_dir, device_ids):
            import jax
            jax.devices()
            if device_ids:
                ids = (ctypes.c_int64 * len(device_ids))(*device_ids)
                rc = lib.axon_start_nrt_profile(ids, len(device_ids))
            else:
                rc = lib.axon_start_nrt_profile(None, 0)
            if rc != 0:
                raise RuntimeError(f"axon_start_nrt_profile rc={rc}")
            try:
                yield
            finally:
                n = lib.axon_stop_nrt_profile(str(output_dir).encode())
                print(f"profile: {n} file(s) -> {output_dir}", file=sys.stderr)

        return _hook

    hook = _make(so_path)
    mod = types.ModuleType("antenv.axon_hooks")
    mod.get_axon_ntff_profile_hook = lambda: hook
    mod.set_axon_ntff_profile_hook = lambda h: None
    sys.modules["antenv.axon_hooks"] = mod
    import concourse.bass_utils as bu
    bu.upload_artifacts = lambda tmpdir: tmpdir


# ---------------------------------------------------------------- IR builder
def _build(flags):
    import concourse.bass as bass
    import concourse.mybir as mybir
    import concourse.tile as tile
    from concourse import bacc
    from concourse.masks import make_identity

    dt = mybir.dt
    AF = mybir.ActivationFunctionType
    OP = mybir.AluOpType
    DRM = mybir.MatmulPerfMode.DoubleRow
    f32, bf16, f8 = dt.float32, dt.bfloat16, dt.float8e4

    nc = bacc.Bacc("TRN2", target_bir_lowering=False, debug=False,
                   enable_asserts=False, num_devices=NCORES)

    x_d = nc.dram_tensor("x", [16, P, H], f32, kind="ExternalInput").ap()
    wq_d = nc.dram_tensor("wq", [8, 8, P, P], f8, kind="ExternalInput").ap()
    wk_d = nc.dram_tensor("wk", [8, 8, P, P], f8, kind="ExternalInput").ap()
    wv_d = nc.dram_tensor("wv", [2, 8, P, 512], f8, kind="ExternalInput").ap()
    wo_d = nc.dram_tensor("wo", [8, P, H], f8, kind="ExternalInput").ap()
    w1_d = nc.dram_tensor("w1", [32, 8, P, P], bf16, kind="ExternalInput").ap()
    w2_d = nc.dram_tensor("w2", [32, P, H], bf16, kind="ExternalInput").ap()
    out_d = nc.dram_tensor("out", [8, P, H], f32, kind="ExternalOutput").ap()

    opt_d = {}
    if flags["ln1"]:
        opt_d["nw"] = nc.dram_tensor("nw", [P, H], f32, kind="ExternalInput").ap()
        opt_d["nb"] = nc.dram_tensor("nb", [P, H], f32, kind="ExternalInput").ap()
    if flags["ln2"]:
        opt_d["anw"] = nc.dram_tensor("anw", [P, H], f32, kind="ExternalInput").ap()
        opt_d["anb"] = nc.dram_tensor("anb", [P, H], f32, kind="ExternalInput").ap()
    if flags["bqk"]:
        opt_d["bqk"] = nc.dram_tensor("bqk", [16, P, 1], f32, kind="ExternalInput").ap()
    if flags["bv"]:
        opt_d["bv"] = nc.dram_tensor("bv", [2, P, 512], f32, kind="ExternalInput").ap()
    if flags["bo"]:
        opt_d["bo"] = nc.dram_tensor("bo", [P, H], f32, kind="ExternalInput").ap()
    if flags["b1"]:
        opt_d["b1"] = nc.dram_tensor("b1", [32, P, 1], f32, kind="ExternalInput").ap()
    if flags["b2"]:
        opt_d["b2"] = nc.dram_tensor("b2", [P, H], f32, kind="ExternalInput").ap()
    if flags["mask"]:
        opt_d["mask"] = nc.dram_tensor("mask", [P, 16], f32, kind="ExternalInput").ap()

    with tile.TileContext(nc) as tc:
        es = contextlib.ExitStack()
        with es:
            const = es.enter_context(tc.tile_pool(name="const", bufs=1, side="left"))
            ident = const.tile([P, P], bf16)
            make_identity(nc, ident)
            eps_c = const.tile([P, 1], f32)
            nc.vector.memset(eps_c[:], EPS)

            opt_sb = {}
            for k, ap in opt_d.items():
                t = const.tile(list(ap.shape), f32, name=f"sb_{k}")
                nc.sync.dma_start(t[:], ap[:])
                opt_sb[k] = t

            # ---- PSUM pools: 4 + 2 + 2 = 8 banks ----
            ps_s = es.enter_context(tc.tile_pool(name="ps_s", bufs=1, space="PSUM"))
            ps_pc = es.enter_context(tc.tile_pool(name="ps_pc", bufs=1, space="PSUM"))
            ps_mm = es.enter_context(tc.tile_pool(name="ps_mm", bufs=2, space="PSUM"))

            # ---- persistent attention tensors (left) ----
            pers = es.enter_context(tc.tile_pool(name="pers", bufs=1, side="left"))
            qT8 = pers.tile([P, 8, OWN], f8)
            kT8 = pers.tile([P, 8, TOK], f8)
            vplus8 = pers.tile([P, 16, HEADS * 65], f8)
            wo8 = pers.tile([P, 8, H], f8)
            nc.vector.memset(
                vplus8[:, :, :].rearrange("p a (h c) -> p a h c", h=HEADS)[:, :, :, 64:65],
                1.0)
            for cb in range(8):
                nc.sync.dma_start(wo8[:, cb, :], wo_d[cb])

            scr = es.enter_context(tc.tile_pool(name="scr", bufs=1, side="left"))

            # ---- right stack: staged tensors, closed as phases finish ----
            wv_es = contextlib.ExitStack()
            wv_p = wv_es.enter_context(tc.tile_pool(name="wv", bufs=1, side="right"))
            wv_sb = wv_p.tile([P, 2, 8, 512], f8)
            for nb in range(2):
                nc.sync.dma_start(wv_sb[:, nb], wv_d[nb].rearrange("a p b -> p a b"))

            xlnT_es = contextlib.ExitStack()
            xlnT8 = xlnT_es.enter_context(
                tc.tile_pool(name="xlnT", bufs=1, side="right")).tile(
                [P, 8, TOK], f8, name="xlnT8_t")

            wqk_es = contextlib.ExitStack()
            wqk_p = wqk_es.enter_context(tc.tile_pool(name="wqk", bufs=1, side="right"))
            wq_sb = wqk_p.tile([P, 8, 8, P], f8)
            wk_sb = wqk_p.tile([P, 8, 8, P], f8)
            for mb in range(8):
                nc.sync.dma_start(wq_sb[:, mb], wq_d[mb].rearrange("a p b -> p a b"))
                nc.sync.dma_start(wk_sb[:, mb], wk_d[mb].rearrange("a p b -> p a b"))

            sa_es = contextlib.ExitStack()
            sa_p = sa_es.enter_context(tc.tile_pool(name="sa", bufs=1, side="right"))

            # ---------------- helpers ----------------
            def keepalive():
                """Tiny standalone LDWEIGHTS: keeps the PE activity monitor
                from re-throttling the clock (K=4/8) during scalar-bound
                stretches.  ~53ns, no PSUM, next matmul reloads weights."""
                nc.tensor.ldweights(ident[0:64, 0:64])

            def ln_stats(src_ap, pool, tag):
                """mean/var over free axis (H) of [128, H] -> (rsig, nbias)."""
                stats = pool.tile([P, 12], f32, tag=f"{tag}_st", bufs=2)
                mv = pool.tile([P, 2], f32, tag=f"{tag}_mv", bufs=2)
                nc.vector.bn_stats(stats[:, 0:6], src_ap[:, 0:512])
                nc.vector.bn_stats(stats[:, 6:12], src_ap[:, 512:1024])
                nc.vector.bn_aggr(mv[:], stats[:])
                sig = pool.tile([P, 1], f32, tag=f"{tag}_sig", bufs=2)
                nc.scalar.activation(sig[:], mv[:, 1:2], AF.Sqrt, bias=eps_c[:])
                rsig = pool.tile([P, 1], f32, tag=f"{tag}_rs", bufs=2)
                nc.vector.reciprocal(rsig[:], sig[:])
                nbias = pool.tile([P, 1], f32, tag=f"{tag}_nb", bufs=2)
                nc.vector.scalar_tensor_tensor(
                    nbias[:], mv[:, 0:1], -1.0, rsig[:], OP.mult, OP.mult)
                return rsig, nbias

            def ln_apply(dst_bf16, src_ap, rsig, nbias, w_sb, b_sb, pool, tag):
                # (x * rsig + nbias) on DVE (keeps the scalar engine free
                # for softmax exp)
                if w_sb is None:
                    nc.vector.tensor_scalar(dst_bf16[:], src_ap, rsig[:],
                                            nbias[:], OP.mult, OP.add)
                else:
                    tmp = pool.tile([P, H], f32, tag=f"{tag}_tmp", bufs=2)
                    nc.vector.tensor_scalar(tmp[:], src_ap, rsig[:],
                                            nbias[:], OP.mult, OP.add)
                    if b_sb is None:
                        nc.vector.tensor_tensor(dst_bf16[:], tmp[:], w_sb[:], op=OP.mult)
                    else:
                        nc.vector.tensor_tensor(tmp[:], tmp[:], w_sb[:], op=OP.mult)
                        nc.vector.tensor_tensor(dst_bf16[:], tmp[:], b_sb[:], op=OP.add)

            def transpose_row(src_bf16, dst3, dst_col):
                """src [128tok, 1024feat] bf16 -> dst3[:, fb, dst_col:+128] fb 0..7.

                Goes through a bf16 bitcast view of one fp32 ps_mm tile
                (2 PSUM banks hold all 8 [128,128] transposes)."""
                pt = ps_mm.tile([P, 512], f32, tag="pmm")
                v = pt[:].bitcast(bf16)
                for j in range(8):
                    nc.tensor.transpose(
                        v[:, j * P:(j + 1) * P], src_bf16[:, j * P:(j + 1) * P],
                        ident[:])
                nc.vector.tensor_copy(
                    dst3[:, :, dst_col:dst_col + P],
                    v.rearrange("p (a b) -> p a b", a=8))

            bqk = opt_sb.get("bqk")

            def qk_block(w_sb, n, dstT, bias_mb0):
                """Project one 512-token column block for q or k (fp8 DoubleRow)."""
                for mb in range(8):
                    ps = ps_mm.tile([P, 512], f32, tag="pmm")
                    for k in range(0, 8, 2):
                        nc.tensor.matmul(
                            ps[:], w_sb[:, mb, k:k + 2, :],
                            xlnT8[:, k:k + 2, n * 512:(n + 1) * 512],
                            start=(k == 0), stop=(k == 6), perf_mode=DRM)
                    dst = dstT[:, mb, n * 512:(n + 1) * 512]
                    if bqk is None:
                        nc.vector.tensor_copy(dst, ps[:])
                    else:
                        nc.vector.tensor_scalar(
                            dst, ps[:], bqk[:, bias_mb0 + mb, :], None, OP.add)

            bv = opt_sb.get("bv")

            def v_block(tt):
                """V projection for one 128-token block (fp8 DoubleRow)."""
                for nb in range(2):
                    ps = ps_mm.tile([P, 512], f32, tag="pmm")
                    for k in range(0, 8, 2):
                        nc.tensor.matmul(
                            ps[:], xlnT8[:, k:k + 2, tt * P:(tt + 1) * P],
                            wv_sb[:, nb, k:k + 2, :],
                            start=(k == 0), stop=(k == 6), perf_mode=DRM)
                    dst = vplus8[:, tt, :].rearrange(
                        "p (h c) -> p h c", h=HEADS)[:, nb * 8:(nb + 1) * 8, 0:64]
                    src = ps[:].rearrange("p (h c) -> p h c", h=8)
                    if bv is None:
                        nc.vector.tensor_copy(dst, src)
                    else:
                        nc.vector.tensor_tensor(
                            dst, src,
                            bv[:, nb, :].rearrange("p (h c) -> p h c", h=8), op=OP.add)

            # ---------------- stage A+B: LN1 + transpose + Q/K proj ------
            ln1_w = opt_sb.get("nw")
            ln1_b = opt_sb.get("nb")
            for t in range(16):
                xt = sa_p.tile([P, H], f32, tag="xs", bufs=2)
                nc.sync.dma_start(xt[:], x_d[t])
                keepalive()
                rsig, nbias = ln_stats(xt[:], sa_p, "l1")
                xln = sa_p.tile([P, H], bf16, tag="xln", bufs=2)
                ln_apply(xln, xt[:], rsig, nbias, ln1_w, ln1_b, sa_p, "l1")
                transpose_row(xln, xlnT8, t * P)
                if t == 3:
                    qk_block(wq_sb, 0, qT8, 0)
                    qk_block(wk_sb, 0, kT8, 8)
                elif t == 7:
                    qk_block(wq_sb, 1, qT8, 0)
                    qk_block(wk_sb, 1, kT8, 8)
                elif t == 11:
                    qk_block(wk_sb, 2, kT8, 8)
                elif t == 15:
                    qk_block(wk_sb, 3, kT8, 8)
            sa_es.close()
            wqk_es.close()

            # V projection for all 16 token blocks (every head-pair's ctx
            # accumulation reads the full kv range, so this cannot be
            # deferred into the attention loop).
            for tt in range(16):
                v_block(tt)
            xlnT_es.close()
            wv_es.close()

            # ---------------- attention + FFN pipeline -------------------
            mask_sb = opt_sb.get("mask")
            bo = opt_sb.get("bo")
            b1 = opt_sb.get("b1")
            b2 = opt_sb.get("b2")
            ln2_w = opt_sb.get("anw")
            ln2_b = opt_sb.get("anb")

            lazy_pools = {}

            def pool(name, side="left"):
                if name not in lazy_pools:
                    lazy_pools[name] = es.enter_context(
                        tc.tile_pool(name=name, bufs=1, side=side))
                return lazy_pools[name]

            # Two filler queues, drained between attention matmuls of the
            # NEXT chunk.  fill_q must be empty before that chunk's O-proj
            # (its thunks read the previous chunk's ao/ylnT tiles, and a
            # later ao write waiting on a slot would deadlock the in-order
            # DVE queue).  late_q (FFN2 groups 1-3: only acc/w2g/outt) may
            # keep filling the O-proj / LN2 / chunk-tail stretch.
            fill_q = []
            late_q = []

            def drain(k):
                for _ in range(k):
                    if fill_q:
                        fill_q.pop(0)()
                    elif late_q:
                        late_q.pop(0)()
                    else:
                        break

            def drain_fill():
                while fill_q:
                    fill_q.pop(0)()

            def attn_chunk(ci, first):
                qo = sum(CHUNKS[:ci])
                qw = CHUNKS[ci]
                nmb = qw // P
                mb0 = qo // P
                # residual x reload for this chunk
                xch = []
                for j in range(nmb):
                    xt = pool("xch").tile([P, H], f32, tag="xch", bufs=2)
                    nc.sync.dma_start(xt[:], x_d[mb0 + j])
                    xch.append(xt)
                ctx8 = pool("ctx8").tile([P, 8, 512], f8, tag="ctx8", bufs=2)
                for hb in range(8):
                    pcA = ps_pc.tile([P, 512], f32, tag="pcA")
                    pcB = ps_pc.tile([P, 512], f32, tag="pcB")
                    hA, hB = 2 * hb, 2 * hb + 1
                    for p in range(8):
                        # Emission order matters: scoresA -> expA -> scoresB
                        # -> expB lets each exp overlap the other head's
                        # score matmuls (a single batched exp strictly
                        # alternates PE and ACT via the shared PSUM tile).
                        sA = ps_s.tile([P, 2, 512], f32, tag="sA")
                        prA = pool("probs").tile([P, 2, 512], f8, tag="prA",
                                                 bufs=2)
                        for i in range(2):
                            kc = 2 * p + i
                            nc.tensor.matmul(
                                sA[:, i, 0:qw], kT8[0:64, hb, kc * P:(kc + 1) * P],
                                qT8[0:64, hb, qo:qo + qw], start=True, stop=True)
                        if mask_sb is None:
                            nc.scalar.activation(prA[:, :, 0:qw], sA[:, :, 0:qw],
                                                 AF.Exp, scale=SCALE)
                        else:
                            for i in range(2):
                                m = mask_sb[:, 2 * p + i:2 * p + i + 1]
                                nc.scalar.activation(prA[:, i, 0:qw],
                                                     sA[:, i, 0:qw],
                                                     AF.Exp, bias=m, scale=SCALE)
                        sB = ps_s.tile([P, 2, 512], f32, tag="sB")
                        prB = pool("probs").tile([P, 2, 512], f8, tag="prB",
                                                 bufs=2)
                        for i in range(2):
                            kc = 2 * p + i
                            nc.tensor.matmul(
                                sB[:, i, 0:qw],
                                kT8[64:128, hb, kc * P:(kc + 1) * P],
                                qT8[64:128, hb, qo:qo + qw], start=True, stop=True)
                        if mask_sb is None:
                            nc.scalar.activation(prB[:, :, 0:qw], sB[:, :, 0:qw],
                                                 AF.Exp, scale=SCALE)
                        else:
                            for i in range(2):
                                m = mask_sb[:, 2 * p + i:2 * p + i + 1]
                                nc.scalar.activation(prB[:, i, 0:qw],
                                                     sB[:, i, 0:qw],
                                                     AF.Exp, bias=m, scale=SCALE)
                        drain(1)
                        nc.tensor.matmul(
                            pcA[0:65, 0:qw],
                            vplus8[:, 2 * p:2 * p + 2, hA * 65:(hA + 1) * 65],
                            prA[:, 0:2, 0:qw],
                            start=(p == 0), stop=(p == 7), perf_mode=DRM)
                        nc.tensor.matmul(
                            pcB[0:65, 0:qw],
                            vplus8[:, 2 * p:2 * p + 2, hB * 65:(hB + 1) * 65],
                            prB[:, 0:2, 0:qw],
                            start=(p == 0), stop=(p == 7), perf_mode=DRM)
                    # Drain pc to SBUF immediately (cheap DVE copies) so the
                    # next head pair's ctx accumulation isn't serialized
                    # behind the reciprocal chain (that serialization stalls
                    # the PE >3.4us and re-throttles the HAM clock gate).
                    # Both denominators go through ONE [2,512] reciprocal --
                    # a [1,512] DVE reciprocal is lane-starved (~4us).
                    crA = scr.tile([65, 512], bf16, tag="crA", bufs=2)
                    crB = scr.tile([65, 512], bf16, tag="crB", bufs=2)
                    nc.vector.tensor_copy(crA[:, 0:qw], pcA[0:65, 0:qw])
                    nc.vector.tensor_copy(crB[:, 0:qw], pcB[0:65, 0:qw])
                    rrA = scr.tile([1, 512], f32, tag="rrA", bufs=2)
                    rrB = scr.tile([1, 512], f32, tag="rrB", bufs=2)
                    nc.vector.reciprocal(rrA[:, 0:qw], crA[64:65, 0:qw])
                    nc.vector.reciprocal(rrB[:, 0:qw], crB[64:65, 0:qw])
                    rbA = scr.tile([64, 512], f32, tag="rbA", bufs=2)
                    rbB = scr.tile([64, 512], f32, tag="rbB", bufs=2)
                    nc.gpsimd.partition_broadcast(rbA[:, 0:qw], rrA[:, 0:qw],
                                                  channels=64)
                    nc.gpsimd.partition_broadcast(rbB[:, 0:qw], rrB[:, 0:qw],
                                                  channels=64)
                    nc.vector.tensor_tensor(ctx8[0:64, hb, 0:qw], crA[0:64, 0:qw],
                                            rbA[:, 0:qw], op=OP.mult)
                    nc.vector.tensor_tensor(ctx8[64:128, hb, 0:qw], crB[0:64, 0:qw],
                                            rbB[:, 0:qw], op=OP.mult)
                drain_fill()
                # ---- O projection + residual, LN2, ylnT ----
                aos = []
                ylnT = pool("ylnT").tile([P, 8, 512], bf16, tag="ylnT", bufs=1)
                for j in range(nmb):
                    ao = pool("ao").tile([P, H], bf16, tag="ao", bufs=4)
                    for nb in range(2):
                        ps = ps_mm.tile([P, 512], f32, tag="pmm")
                        for cb in range(0, 8, 2):
                            nc.tensor.matmul(
                                ps[:], ctx8[:, cb:cb + 2, j * P:(j + 1) * P],
                                wo8[:, cb:cb + 2, nb * 512:(nb + 1) * 512],
                                start=(cb == 0), stop=(cb == 6), perf_mode=DRM)
                        dst = ao[:, nb * 512:(nb + 1) * 512]
                        xs = xch[j][:, nb * 512:(nb + 1) * 512]
                        if bo is None:
                            nc.vector.scalar_tensor_tensor(
                                dst, ps[:], OSC, xs, OP.mult, OP.add)
                        else:
                            tmp = scr.tile([P, 512], f32, tag="botmp", bufs=2)
                            nc.vector.scalar_tensor_tensor(
                                tmp[:], ps[:], OSC, xs, OP.mult, OP.add)
                            nc.vector.tensor_tensor(
                                dst, tmp[:], bo[:, nb * 512:(nb + 1) * 512],
                                op=OP.add)
                    aos.append(ao)
                    keepalive()
                    rsig, nbias = ln_stats(ao[:], scr, "l2")
                    yln = scr.tile([P, H], bf16, tag="yln", bufs=2)
                    ln_apply(yln, ao[:], rsig, nbias, ln2_w, ln2_b, scr, "l2")
                    transpose_row(yln, ylnT, j * P)
                    drain(6)
                drain(len(fill_q) + len(late_q))   # finish previous chunk's FFN
                return ylnT, aos, qw, nmb, mb0

            def enqueue_ffn(ylnT, aos, qw, nmb, mb0):
                """Queue the whole FFN for one chunk as filler thunks, drained
                between score/ctx matmuls of the NEXT chunk's attention:
                FFN1 matmuls -> pre-gelu hT (bf16) -> one gelu burst (batched
                to amortize act-table switches) -> FFN2 with SBUF partial
                accumulation (W2 streamed in 8-block groups)."""
                state = {}
                hts = []

                def w1_dma(fb):
                    def run():
                        w1t = pool("w1s").tile([P, 8, P], bf16, tag="w1t", bufs=3,
                                               name="w1t_t")
                        nc.sync.dma_start(w1t[:], w1_d[fb].rearrange("a p b -> p a b"))
                        state[("w1", fb)] = w1t
                    return run

                def f1(g, i):
                    fb = 8 * g + i
                    def run():
                        if i == 0:
                            ht = pool("hT").tile([P, 8, 512], bf16, tag="hT",
                                                 bufs=4, name="hT_t")
                            hts.append(ht)
                        ht = hts[g]
                        ps = ps_mm.tile([P, 512], f32, tag="pmm", name="ps_f1")
                        for k in range(8):
                            nc.tensor.matmul(
                                ps[:, 0:qw], state[("w1", fb)][:, k, :],
                                ylnT[:, k, 0:qw],
                                start=(k == 0), stop=(k == 7))
                        del state[("w1", fb)]
                        # psum -> SBUF on the scalar engine: Copy shares a
                        # table set with Exp (no act-table switch) and this
                        # keeps DVE free for the normalize/LN work
                        nc.scalar.copy(ht[:, i, 0:qw], ps[:, 0:qw])
                    return run

                def gelu_burst():
                    for g in range(4):
                        if b1 is None:
                            nc.scalar.activation(hts[g][:, :, 0:qw],
                                                 hts[g][:, :, 0:qw], AF.Gelu)
                        else:
                            for i in range(8):
                                nc.scalar.activation(
                                    hts[g][:, i, 0:qw], hts[g][:, i, 0:qw],
                                    AF.Gelu, bias=b1[:, 8 * g + i, :])

                def w2_dma(g):
                    def run():
                        w2g = pool("w2s").tile([P, 8, H], bf16, tag="w2g", bufs=2,
                                               name="w2g_t")
                        for i in range(8):
                            nc.sync.dma_start(w2g[:, i, :], w2_d[8 * g + i])
                        state[g] = w2g
                    return run

                def f2(g, j, nb):
                    def run():
                        w2g = state[g]
                        ht = hts[g]
                        ps = ps_mm.tile([P, 512], f32, tag="pmm")
                        for i in range(8):
                            nc.tensor.matmul(
                                ps[:], ht[:, i, j * P:(j + 1) * P],
                                w2g[:, i, nb * 512:(nb + 1) * 512],
                                start=(i == 0), stop=(i == 7))
                        ns = slice(nb * 512, (nb + 1) * 512)
                        if g == 0:
                            if ("acc", j) not in state:
                                state[("acc", j)] = pool("acc").tile(
                                    [P, H], bf16, tag="acc", bufs=4, name="acc_t")
                            nc.vector.tensor_tensor(
                                state[("acc", j)][:, ns], ps[:], aos[j][:, ns],
                                op=OP.add)
                        elif g < 3:
                            acc = state[("acc", j)]
                            nc.vector.tensor_tensor(acc[:, ns], acc[:, ns], ps[:],
                                                    op=OP.add)
                        else:
                            if ("out", j) not in state:
                                state[("out", j)] = pool("outp").tile(
                                    [P, H], f32, tag="outt", bufs=1, name="outt_t")
                            outt = state[("out", j)]
                            nc.vector.tensor_tensor(outt[:, ns],
                                                    state[("acc", j)][:, ns],
                                                    ps[:], op=OP.add)
                            if b2 is not None:
                                nc.vector.tensor_tensor(outt[:, ns], outt[:, ns],
                                                        b2[:, ns], op=OP.add)
                            if nb == 1:
                                nc.sync.dma_start(out_d[mb0 + j], outt[:])
                    return run

                fill_q.append(w1_dma(0))
                fill_q.append(w1_dma(1))
                for g in range(4):
                    for i in range(8):
                        fb = 8 * g + i
                        if fb + 2 < 32:
                            fill_q.append(w1_dma(fb + 2))
                        fill_q.append(f1(g, i))
                fill_q.append(gelu_burst)
                fill_q.append(w2_dma(0))
                fill_q.append(w2_dma(1))
                for g in range(4):
                    q = fill_q if g == 0 else late_q
                    if g >= 2:
                        q.append(w2_dma(g))
                    for j in range(nmb):
                        for nb in range(2):
                            q.append(f2(g, j, nb))

            # ---- run the pipeline ----
            for ci in range(len(CHUNKS)):
                ylnT, aos, qw, nmb, mb0 = attn_chunk(ci, first=(ci == 0))
                enqueue_ffn(ylnT, aos, qw, nmb, mb0)
            drain(len(fill_q) + len(late_q))

    nc.compile()
    return nc


# ---------------------------------------------------------------- host side
def _prep_weights(qkv_w, attn_ow, inter_w, output_w):
    bf = ml_dtypes.bfloat16
    f8 = ml_dtypes.float8_e4m3
    q8 = lambda a: np.ascontiguousarray(
        np.clip(a * WS, -240.0, 240.0)).astype(f8)
    wq = q8(qkv_w[:, 0:1024].reshape(8, P, 8, P).transpose(2, 0, 1, 3))
    wk = q8(qkv_w[:, 1024:2048].reshape(8, P, 8, P).transpose(2, 0, 1, 3))
    wv = q8(qkv_w[:, 2048:3072].reshape(8, P, 2, 512).transpose(2, 0, 1, 3))
    wo = q8(attn_ow.reshape(8, P, H))
    w1 = np.ascontiguousarray(
        inter_w.reshape(8, P, 32, P).transpose(2, 0, 1, 3)).astype(bf)
    w2 = np.ascontiguousarray(output_w.reshape(32, P, H)).astype(bf)
    return wq, wk, wv, wo, w1, w2


def kernel(input, input_mask, norm_w, norm_b, qkv_w, qkv_b, attn_ow, attn_ob,
           attn_nw, attn_nb, inter_w, inter_b, output_w, output_b):
    global LAST_EXEC_NS
    _install_trace_hook()
    from concourse.bass_utils import run_bass_kernel_spmd

    input = np.asarray(input, dtype=np.float32)
    input_mask = np.asarray(input_mask, dtype=np.float32)
    f32 = lambda a: np.asarray(a, dtype=np.float32)
    norm_w, norm_b = f32(norm_w), f32(norm_b)
    qkv_b, attn_ob = f32(qkv_b), f32(attn_ob)
    attn_nw, attn_nb = f32(attn_nw), f32(attn_nb)
    inter_b, output_b = f32(inter_b), f32(output_b)

    flags = {
        "ln1": not (np.all(norm_w == 1.0) and np.all(norm_b == 0.0)),
        "ln2": not (np.all(attn_nw == 1.0) and np.all(attn_nb == 0.0)),
        "bqk": bool(np.any(qkv_b[0:2048] != 0.0)),
        "bv": bool(np.any(qkv_b[2048:3072] != 0.0)),
        "bo": bool(np.any(attn_ob != 0.0)),
        "b1": bool(np.any(inter_b != 0.0)),
        "b2": bool(np.any(output_b != 0.0)),
        "mask": bool(np.any(input_mask != 0.0)),
    }
    key = tuple(sorted(flags.items()))
    if key not in _CACHE:
        _CACHE[key] = _build(flags)
    nc = _CACHE[key]

    wq, wk, wv, wo, w1, w2 = _prep_weights(
        f32(qkv_w), f32(attn_ow), f32(inter_w), f32(output_w))

    common = {"wq": wq, "wk": wk, "wv": wv, "wo": wo, "w1": w1, "w2": w2}
    bcast = lambda v: np.ascontiguousarray(
        np.broadcast_to(v.reshape(1, H), (P, H)).astype(np.float32))
    if flags["ln1"]:
        common["nw"] = bcast(norm_w); common["nb"] = bcast(norm_b)
    if flags["ln2"]:
        common["anw"] = bcast(attn_nw); common["anb"] = bcast(attn_nb)
    if flags["bqk"]:
        common["bqk"] = np.ascontiguousarray(
            (qkv_b[0:2048] * WS).reshape(16, P, 1).astype(np.float32))
    if flags["bv"]:
        common["bv"] = np.ascontiguousarray(
            (qkv_b[2048:3072] * WS).reshape(2, 512)[:, None, :]
            .repeat(P, 1).astype(np.float32))
    if flags["bo"]:
        common["bo"] = bcast(attn_ob)
    if flags["b1"]:
        common["b1"] = np.ascontiguousarray(
            inter_b.reshape(32, P, 1).astype(np.float32))
    if flags["b2"]:
        common["b2"] = bcast(output_b)

    in_maps = []
    for c in range(NCORES):
        b, r = c // 2, (c % 2) * OWN
        xb = np.roll(input[b], -r, axis=0) if r else input[b]
        m = dict(common)
        m["x"] = np.ascontiguousarray(xb.reshape(16, P, H))
        if flags["mask"]:
            mk = input_mask[b, 0, 0]
            mk = np.roll(mk, -r) if r else mk
            m["mask"] = np.ascontiguousarray(
                mk.reshape(16, P).T.astype(np.float32))
        in_maps.append(m)

    trace = bool(os.environ.get("BASS_TRACE"))
    res = run_bass_kernel_spmd(nc, in_maps, list(range(NCORES)), trace=trace)
    LAST_EXEC_NS = res.exec_time_ns
    if res.exec_time_ns is not None:
        print(f"HW exec time: {res.exec_time_ns} ns")

    out = np.empty((B, S, H), dtype=np.float32)
    for c in range(NCORES):
        b, r = c // 2, (c % 2) * OWN
        out[b, r:r + OWN] = res.results[c]["out"].reshape(OWN, H)
    return out
